# revision 1
# baseline (speedup 1.0000x reference)
"""AFNO2D Trainium2 kernel: rfft2 -> block-diag complex MLP -> irfft2 (+x on host).

Self-contained. Strategy:
- Data-parallel over batch: core i processes sample i (B=8 == 8 cores). No collectives.
- FFTs via DFT matmuls in bf16 (fp32 PSUM accumulation).
- Per core: loop over 8 channel blocks (96 ch); whole pipeline for one block
  lives in SBUF. Orientation changes are folded into the matmuls by making the
  DATA the stationary operand (strided lhsT access patterns) at stages where
  the contraction axis changes, so no explicit transposes are needed.
- W-axis real FFT packed into an orthogonal-ish 128x128 real matrix
  (65 Re rows | 63 Im rows); irfft ignores Im at modes 0,64 which the packed
  inverse matrix reproduces exactly.
- Residual add (+x) and final f32 cast are done on the host (exact, and the
  device output is only the small FFT-path correction, so bf16 output is safe).
"""
import sys
import numpy as np

sys.path.insert(0, "/opt/trn_rl_repo")

H = 128
W = 128
C = 768
NB = 8
BS = 96
WF = 65
LAM = 0.01
NCORES = 8
FREE = H * BS          # 12288 free size of [128, (h,c)]-style tiles
POS = WF * H           # 8320 MLP positions per block
ZCOLS = WF * BS        # 6240


def _dft_mats():
    n = 128
    k = np.arange(n)
    w = np.arange(n)
    ang = 2.0 * np.pi * np.outer(w, k) / n
    c = np.cos(ang) / np.sqrt(n)
    s = np.sin(ang) / np.sqrt(n)
    fw = np.concatenate([c[:, :65], -s[:, 1:64]], axis=1)   # [w, kp]
    ch = c                                                   # [h, m] (symmetric)
    sh = s
    gw = np.zeros((n, n))
    gw[0, :] = 1.0 / np.sqrt(n)
    gw[64, :] = c[:, 64]
    gw[1:64, :] = 2.0 * c[:, 1:64].T
    gw[65:128, :] = -2.0 * s[:, 1:64].T
    return fw, ch, sh, gw


def _build_graph(rep=1):
    from contextlib import ExitStack
    from concourse import bass, bacc, tile, mybir

    bf16 = mybir.dt.bfloat16
    f32 = mybir.dt.float32

    nc = bacc.Bacc("TRN2", target_bir_lowering=False, debug=False,
                   num_devices=NCORES)

    xin = nc.dram_tensor("x", [NB, W, FREE], bf16, kind="ExternalInput")
    oext = nc.dram_tensor("out", [NB, W, FREE], bf16, kind="ExternalOutput")
    m_fw = nc.dram_tensor("fw", [128, 128], bf16, kind="ExternalInput")
    m_ch = nc.dram_tensor("ch", [128, 128], bf16, kind="ExternalInput")
    m_sh = nc.dram_tensor("sh", [128, 128], bf16, kind="ExternalInput")
    m_shn = nc.dram_tensor("shn", [128, 128], bf16, kind="ExternalInput")
    m_gwa = nc.dram_tensor("gwa", [65, 128], bf16, kind="ExternalInput")
    m_gwb = nc.dram_tensor("gwb", [64, 128], bf16, kind="ExternalInput")
    w_ext = {}
    for nm in ("w1r", "w1i", "w1in", "w2r", "w2i", "w2in"):
        w_ext[nm] = nc.dram_tensor(nm, [NB, BS, BS], bf16, kind="ExternalInput")
    b1_ext = nc.dram_tensor("b1", [NB, 2, BS, 1], f32, kind="ExternalInput")
    b2_ext = nc.dram_tensor("b2", [NB, 1, 2 * BS], bf16, kind="ExternalInput")

    RELU = mybir.ActivationFunctionType.Relu
    SUB = mybir.AluOpType.subtract
    ADD = mybir.AluOpType.add

    # L1 free-dim chunking (PSUM bank = 512 f32)
    l1_chunks = [(i * 512, 512) for i in range(16)] + [(8192, 128)]
    inv_chunks = [(i * 512, 512) for i in range(24)]

    with tile.TileContext(nc) as tc, ExitStack() as ctx:
        const = ctx.enter_context(tc.tile_pool(name="const", bufs=1))
        wpool = ctx.enter_context(tc.tile_pool(name="wp", bufs=2))
        bpool = ctx.enter_context(tc.tile_pool(name="bp", bufs=2))
        pa = ctx.enter_context(tc.tile_pool(name="pa", bufs=3))
        pb = ctx.enter_context(tc.tile_pool(name="pb", bufs=4))
        pcz = ctx.enter_context(tc.tile_pool(name="pc", bufs=2))
        scr = ctx.enter_context(tc.tile_pool(name="scr", bufs=4))
        pp = ctx.enter_context(tc.tile_pool(name="pp", bufs=8, space="PSUM"))

        fw_t = const.tile([128, 128], bf16, tag="m0")
        ch_t = const.tile([128, 128], bf16, tag="m1")
        sh_t = const.tile([128, 128], bf16, tag="m2")
        shn_t = const.tile([128, 128], bf16, tag="m3")
        gwa_t = const.tile([65, 128], bf16, tag="m4")
        gwb_t = const.tile([64, 128], bf16, tag="m4b")
        ones_t = const.tile([1, 128], bf16, tag="m5")
        nc.sync.dma_start(fw_t[:], m_fw[:])
        nc.sync.dma_start(ch_t[:], m_ch[:])
        nc.sync.dma_start(sh_t[:], m_sh[:])
        nc.sync.dma_start(shn_t[:], m_shn[:])
        nc.sync.dma_start(gwa_t[:], m_gwa[:])
        nc.sync.dma_start(gwb_t[:], m_gwb[:])
        nc.vector.memset(ones_t[:], 1.0)

        def emit_block(b):
            # ---- per-block weights/biases ----
            wt = {}
            for nm in ("w1r", "w1i", "w1in", "w2r", "w2in", "w2i"):
                wt[nm] = wpool.tile([BS, BS], bf16, tag=nm, name=f"{nm}_t")
                nc.sync.dma_start(wt[nm][:], w_ext[nm][b])
            b1r_t = bpool.tile([BS, 1], f32, tag="b1r")
            b1i_t = bpool.tile([BS, 1], f32, tag="b1i")
            nc.sync.dma_start(b1r_t[:], b1_ext[b, 0])
            nc.sync.dma_start(b1i_t[:], b1_ext[b, 1])
            b2ri_t = bpool.tile([1, 192], bf16, tag="b2ri")
            nc.sync.dma_start(b2ri_t[:], b2_ext[b])

            # ---- load x block: [w, (h, c)] ----
            xb = pa.tile([128, FREE], bf16, tag="A")
            nc.sync.dma_start(xb[:], xin[b])

            # ---- S1: W-axis packed real FFT (contract w, data stationary) ----
            # out X1 [h, (c, kp)]  col = c*128 + kp
            x1 = pa.tile([128, FREE], bf16, tag="A")
            for c0 in range(0, BS, 4):
                ps = pp.tile([128, 512], f32, tag="ps")
                for j in range(4):
                    nc.tensor.matmul(ps[:, j * 128:(j + 1) * 128],
                                     xb[:, (c0 + j)::BS], fw_t[:],
                                     start=True, stop=True)
                nc.vector.tensor_copy(x1[:, c0 * 128:(c0 + 4) * 128], ps[:])

            # ---- S2: H-axis complex FFT (contract h, data stationary) ----
            # out Yr/Yi [c, (kw, hm)]  col = kw*128 + hm
            yr = pb.tile([BS, POS], bf16, tag="B")
            yi = pb.tile([BS, POS], bf16, tag="B")
            for k in range(WF):
                xr_sl = x1[:, k::128]            # [h, 96c]
                ksl = slice(k * 128, (k + 1) * 128)
                psr = pp.tile([BS, 128], f32, tag="ps")
                psi = pp.tile([BS, 128], f32, tag="ps")
                if k in (0, 64):
                    nc.tensor.matmul(psr[:], xr_sl, ch_t[:], start=True, stop=True)
                    nc.tensor.matmul(psi[:], xr_sl, shn_t[:], start=True, stop=True)
                else:
                    xi_sl = x1[:, (64 + k)::128]
                    nc.tensor.matmul(psr[:], xr_sl, ch_t[:], start=True, stop=False)
                    nc.tensor.matmul(psi[:], xr_sl, shn_t[:], start=True, stop=False)
                    nc.tensor.matmul(psr[:], xi_sl, sh_t[:], start=False, stop=True)
                    nc.tensor.matmul(psi[:], xi_sl, ch_t[:], start=False, stop=True)
                nc.vector.tensor_copy(yr[:, ksl], psr[:])
                nc.scalar.copy(yi[:, ksl], psi[:])

            # ---- L1: block MLP layer 1 (contract c, weight stationary) ----
            o1r = pb.tile([BS, POS], bf16, tag="B")
            o1i = pb.tile([BS, POS], bf16, tag="B")
            for (off, ln) in l1_chunks:
                sl = slice(off, off + ln)
                psr = pp.tile([BS, 512], f32, tag="ps")
                nc.tensor.matmul(psr[:, :ln], wt["w1r"][:], yr[:, sl], start=True, stop=False)
                nc.tensor.matmul(psr[:, :ln], wt["w1in"][:], yi[:, sl], start=False, stop=True)
                nc.scalar.activation(o1r[:, sl], psr[:, :ln], RELU, bias=b1r_t[:])
                psi = pp.tile([BS, 512], f32, tag="ps")
                nc.tensor.matmul(psi[:, :ln], wt["w1r"][:], yi[:, sl], start=True, stop=False)
                nc.tensor.matmul(psi[:, :ln], wt["w1i"][:], yr[:, sl], start=False, stop=True)
                nc.scalar.activation(o1i[:, sl], psi[:, :ln], RELU, bias=b1i_t[:])

            # ---- L2 (contract hid, data stationary per kw) + bias + softshrink ----
            # psum [128,192]: cols 0:96 = o2r(kw), 96:192 = o2i(kw).
            # Stored Z = clip(v) - v = -softshrink(v); sign folded into gwa/gwb.
            zr = pcz.tile([128, ZCOLS], bf16, tag="C")
            zi = pcz.tile([128, ZCOLS], bf16, tag="C")
            for k in range(WF):
                o1sl = slice(k * 128, (k + 1) * 128)
                zsl = slice(k * BS, (k + 1) * BS)
                psr = pp.tile([128, BS], f32, tag="ps")
                psi = pp.tile([128, BS], f32, tag="ps")
                nc.tensor.matmul(psr[:], o1r[:, o1sl], wt["w2r"][:], start=True, stop=False)
                nc.tensor.matmul(psi[:], o1r[:, o1sl], wt["w2i"][:], start=True, stop=False)
                nc.tensor.matmul(psr[:], o1i[:, o1sl], wt["w2in"][:], start=False, stop=False)
                nc.tensor.matmul(psi[:], o1i[:, o1sl], wt["w2r"][:], start=False, stop=False)
                nc.tensor.matmul(psr[:], ones_t[:], b2ri_t[:, 0:BS], start=False, stop=True)
                nc.tensor.matmul(psi[:], ones_t[:], b2ri_t[:, BS:192], start=False, stop=True)
                uu = scr.tile([128, 192], f32, tag="t1")
                nc.vector.tensor_scalar(uu[:, 0:BS], psr[:], -LAM, LAM,
                                        mybir.AluOpType.max, mybir.AluOpType.min)
                nc.vector.tensor_scalar(uu[:, BS:192], psi[:], -LAM, LAM,
                                        mybir.AluOpType.max, mybir.AluOpType.min)
                nc.vector.tensor_tensor(zr[:, zsl], uu[:, 0:BS], psr[:], SUB)
                nc.vector.tensor_tensor(zi[:, zsl], uu[:, BS:192], psi[:], SUB)

            # ---- invH (contract hm, data stationary per c) ----
            # ps [65,256]: cols 0:128 zr-part (Z@ch + Zi@shn), 128:256 zi-part (Z@sh + Zi@ch)
            # zpa [65,(h,c)] = zr(kw 0..64); zpb [64,(h,c)] = zi(kw 0..63, kw0 zeroed via gwb)
            zpa = pa.tile([65, FREE], bf16, tag="A")
            zpb = pa.tile([64, FREE], bf16, tag="Ab", bufs=1)
            for cc in range(BS):
                zr_sl = zr[:, cc::BS]
                zi_sl = zi[:, cc::BS]
                psa = pp.tile([65, 128], f32, tag="ps")
                psb = pp.tile([65, 128], f32, tag="ps")
                nc.tensor.matmul(psa[:], zr_sl, ch_t[:], start=True, stop=False)
                nc.tensor.matmul(psb[:], zr_sl, sh_t[:], start=True, stop=False)
                nc.tensor.matmul(psa[:], zi_sl, shn_t[:], start=False, stop=True)
                nc.tensor.matmul(psb[:], zi_sl, ch_t[:], start=False, stop=True)
                nc.vector.tensor_copy(zpa[:, cc::BS], psa[:])
                nc.scalar.copy(zpb[:, cc::BS], psb[0:64, :])

            # ---- invW (contract kp split 65+63, DFT stationary) + store ----
            ot = pa.tile([128, FREE], bf16, tag="A")
            for (off, ln) in inv_chunks:
                sl = slice(off, off + ln)
                ps = pp.tile([128, 512], f32, tag="ps")
                nc.tensor.matmul(ps[:], gwa_t[:], zpa[:, sl], start=True, stop=False)
                nc.tensor.matmul(ps[:], gwb_t[:], zpb[:, sl], start=False, stop=True)
                nc.vector.tensor_copy(ot[:, sl], ps[:])
            nc.sync.dma_start(oext[b], ot[:])

        if rep > 1:
            with tc.For_i(0, rep, 1):
                for b in range(NB):
                    emit_block(b)
        else:
            for b in range(NB):
                emit_block(b)

    nc.compile()
    return nc


_COMPILED = None


def _get_compiled():
    global _COMPILED
    if _COMPILED is None:
        _COMPILED = _build_graph()
    return _COMPILED


def _host_inputs(x, w1, b1, w2, b2):
    """Build the per-core in_maps."""
    import ml_dtypes
    bf = ml_dtypes.bfloat16
    fw, ch, sh, gw = _dft_mats()
    shn = -sh
    common = {
        "fw": fw.astype(bf), "ch": ch.astype(bf), "sh": sh.astype(bf),
        "shn": shn.astype(bf),
        # Z holds -softshrink(o2); negate the inverse-W matrix to compensate
        "gwa": (-gw[:65]).astype(bf),
        "gwb": (-np.concatenate([np.zeros((1, 128)), gw[65:]], axis=0)).astype(bf),
        "w1r": np.ascontiguousarray(w1[0]).astype(bf),
        "w1i": np.ascontiguousarray(w1[1]).astype(bf),
        "w1in": np.ascontiguousarray(-w1[1]).astype(bf),
        "w2r": np.ascontiguousarray(w2[0]).astype(bf),
        "w2i": np.ascontiguousarray(w2[1]).astype(bf),
        "w2in": np.ascontiguousarray(-w2[1]).astype(bf),
        "b1": np.ascontiguousarray(b1.transpose(1, 0, 2))[:, :, :, None].astype(np.float32),
        "b2": np.ascontiguousarray(b2.transpose(1, 0, 2).reshape(NB, 1, 2 * BS)).astype(bf),
    }
    in_maps = []
    for i in range(NCORES):
        xi = x[i].reshape(H, W, NB, BS).transpose(2, 1, 0, 3)  # [nb, w, h, bs]
        xi = np.ascontiguousarray(xi).reshape(NB, W, FREE).astype(bf)
        m = dict(common)
        m["x"] = xi
        in_maps.append(m)
    return in_maps


def kernel(x, w1, b1, w2, b2, _trace=False):
    from concourse.bass_utils import run_bass_kernel_spmd

    nc = _get_compiled()
    in_maps = _host_inputs(x, w1, b1, w2, b2)
    res = run_bass_kernel_spmd(nc, in_maps, core_ids=list(range(NCORES)),
                               trace=_trace)
    y = np.empty((NCORES, H, W, C), dtype=np.float32)
    for i in range(NCORES):
        o = np.asarray(res.results[i]["out"]).astype(np.float32)
        o = o.reshape(NB, W, H, BS).transpose(2, 1, 0, 3).reshape(H, W, C)
        y[i] = o + x[i]
    if _trace:
        return y, res
    return y


def _bench_nc(nc, inputs, iters=10):
    """Min wall-clock (ns) of the jitted sharded call for a prebuilt graph."""
    import time
    import jax
    import numpy as np
    from jax.sharding import Mesh, PartitionSpec
    from jax.experimental.shard_map import shard_map
    from concourse import bass2jax, mybir

    bass2jax.install_neuronx_cc_hook()
    in_maps = _host_inputs(inputs["x"], inputs["w1"], inputs["b1"],
                           inputs["w2"], inputs["b2"])

    pname = nc.partition_id_tensor.name if nc.partition_id_tensor else None
    in_names, out_names, out_avals, zero_outs = [], [], [], []
    for alloc in nc.m.functions[0].allocations:
        if not isinstance(alloc, mybir.MemoryLocationSet):
            continue
        name = alloc.memorylocations[0].name
        if alloc.kind == "ExternalInput":
            if name != pname:
                in_names.append(name)
        elif alloc.kind == "ExternalOutput":
            shape = tuple(alloc.tensor_shape)
            dtype = mybir.dt.np(alloc.dtype)
            out_names.append(name)
            out_avals.append(jax.core.ShapedArray(shape, dtype))
            zero_outs.append(np.zeros(shape, dtype))
    n_params = len(in_names)
    all_names = in_names + out_names
    if pname is not None:
        all_names = all_names + [pname]

    def _body(*args):
        operands = list(args)
        if pname is not None:
            operands.append(bass2jax.partition_id_tensor())
        outs = bass2jax._bass_exec_p.bind(
            *operands, out_avals=tuple(out_avals), in_names=tuple(all_names),
            out_names=tuple(out_names), lowering_input_output_aliases=(),
            sim_require_finite=True, sim_require_nnan=True, nc=nc)
        return tuple(outs)

    devices = jax.devices()[:NCORES]
    mesh = Mesh(np.asarray(devices), ("core",))
    nops = n_params + len(out_names)
    sharded = jax.jit(shard_map(_body, mesh=mesh,
                                in_specs=(PartitionSpec("core"),) * nops,
                                out_specs=(PartitionSpec("core"),) * len(out_names),
                                check_rep=False), keep_unused=True)
    concat_in = [np.concatenate([np.asarray(in_maps[c][n]) for c in range(NCORES)], axis=0)
                 for n in in_names]
    concat_zero = [np.zeros((NCORES * z.shape[0], *z.shape[1:]), z.dtype) for z in zero_outs]
    sharding = jax.sharding.NamedSharding(mesh, PartitionSpec("core"))
    dev_in = [jax.device_put(a, sharding) for a in concat_in + concat_zero]
    # warmup (compiles + caches)
    for _ in range(2):
        r = sharded(*dev_in)
        jax.block_until_ready(r)
    best = float("inf")
    for _ in range(iters):
        t0 = time.perf_counter()
        r = sharded(*dev_in)
        jax.block_until_ready(r)
        best = min(best, time.perf_counter() - t0)
    return best * 1e9


def bench(inputs, iters=10, rep=17):
    """Estimate HW exec time via on-device repeat loop slope:
    (T(rep) - T(1)) / (rep - 1)."""
    t1 = _bench_nc(_get_compiled(), inputs, iters)
    ncr = _build_graph(rep=rep)
    tr = _bench_nc(ncr, inputs, iters)
    print(f"  [bench] T(1)={t1/1e6:.2f} ms  T({rep})={tr/1e6:.2f} ms")
    return (tr - t1) / (rep - 1)


if __name__ == "__main__":
    nc = _get_compiled()
    print("graph built + compiled OK")



# revision 26
# speedup vs baseline: 1.5441x; 1.5441x over previous
"""AFNO2D Trainium2 kernel: rfft2 -> block-diag complex MLP -> irfft2 (+x on host).

Self-contained. Strategy (v2):
- Data-parallel over batch: core i processes sample i (B=8 == 8 cores).
- FFT/MLP matmuls in fp8 (e4m3) with DoubleRow perf mode: each complex
  cross-term accumulation pair becomes ONE matmul with a virtual 256-deep
  contraction at 0.5 cycles/row.
- Weights scaled (w1*32, w2*32, b2*1024, lambda*1024, gw/1024) so fp8
  operands sit in range; the residual +x (added on host in f32) damps the
  FFT-path error ~30x relative to the output norm.
- PSUM evacuation spread across DVE / Activation / Pool engines using
  2-bank (1024 f32) psum groups for few, wide copies.
"""
import sys
import numpy as np

sys.path.insert(0, "/opt/trn_rl_repo")

H = 128
W = 128
C = 768
NB = 8
BS = 96
WF = 65
LAM = 0.01
NCORES = 8
SC1 = 32.0
SC2 = 4.0
SCL2 = SC1 * SC2        # 1024
OSCALE = 16.0           # device output pre-scale (avoids fp8 denormals)
FREE = H * BS           # 12288
KPAD = 80               # padded kw stride in x1 (%16 for DR weight APs)


def _dft_mats():
    n = 128
    k = np.arange(n)
    w = np.arange(n)
    ang = 2.0 * np.pi * np.outer(w, k) / n
    c = np.cos(ang) / np.sqrt(n)
    s = np.sin(ang) / np.sqrt(n)
    fw = np.concatenate([c[:, :65], -s[:, 1:64]], axis=1)   # [w, kp] 65re|63im
    gw = np.zeros((n, n))
    gw[0, :] = 1.0 / np.sqrt(n)
    gw[64, :] = c[:, 64]
    gw[1:64, :] = 2.0 * c[:, 1:64].T
    gw[65:128, :] = -2.0 * s[:, 1:64].T
    return fw, c, s, gw


def _build_graph(rep=1):
    from contextlib import ExitStack
    from concourse import bass, bacc, tile, mybir

    bf16 = mybir.dt.bfloat16
    f32 = mybir.dt.float32
    fp8 = mybir.dt.float8e4
    DR = mybir.MatmulPerfMode.DoubleRow
    SUB = mybir.AluOpType.subtract
    ADD = mybir.AluOpType.add
    MAX = mybir.AluOpType.max
    MIN = mybir.AluOpType.min
    SUBR = mybir.AluOpType.subtract
    RELU = mybir.ActivationFunctionType.Relu
    LAMS = LAM * SCL2

    nc = bacc.Bacc("TRN2", target_bir_lowering=False, debug=False,
                   num_devices=NCORES)

    xin = nc.dram_tensor("x", [NB, 64, 96, 2, 128], fp8, kind="ExternalInput")
    oext = nc.dram_tensor("out", [NB, 128, 96, 128], fp8, kind="ExternalOutput")
    m_fw2 = nc.dram_tensor("fw2", [64, 2, 128], fp8, kind="ExternalInput")
    m_chsh = nc.dram_tensor("chsh", [128, 2, 256], fp8, kind="ExternalInput")
    m_zpa = nc.dram_tensor("zpaR", [128, 2, 128], fp8, kind="ExternalInput")
    m_zpb = nc.dram_tensor("zpbR", [128, 2, 128], fp8, kind="ExternalInput")
    m_gwa = nc.dram_tensor("gwA", [65, 128], bf16, kind="ExternalInput")
    m_gwb = nc.dram_tensor("gwB", [63, 128], bf16, kind="ExternalInput")
    m_ones = nc.dram_tensor("onesdr", [1, 2, 128], fp8, kind="ExternalInput")
    w1_ext = nc.dram_tensor("w1dr", [NB, 2, 96, 2, 96], fp8, kind="ExternalInput")
    w2_ext = nc.dram_tensor("w2dr", [NB, 96, 2, 192], fp8, kind="ExternalInput")
    b2_ext = nc.dram_tensor("b2dr", [NB, 1, 2, 384], fp8, kind="ExternalInput")
    b1_ext = nc.dram_tensor("b1s", [NB, 2, 96, 1], f32, kind="ExternalInput")

    with tile.TileContext(nc) as tc, ExitStack() as ctx:
        const = ctx.enter_context(tc.tile_pool(name="const", bufs=1))
        wpool = ctx.enter_context(tc.tile_pool(name="wp", bufs=2))
        bpool = ctx.enter_context(tc.tile_pool(name="bp", bufs=2))
        px = ctx.enter_context(tc.tile_pool(name="px", bufs=2))
        px1 = ctx.enter_context(tc.tile_pool(name="px1", bufs=1))
        py = ctx.enter_context(tc.tile_pool(name="py", bufs=2))
        po1 = ctx.enter_context(tc.tile_pool(name="po1", bufs=1))
        pcc = ctx.enter_context(tc.tile_pool(name="pcc", bufs=4))
        pz = ctx.enter_context(tc.tile_pool(name="pz", bufs=2))
        pzp = ctx.enter_context(tc.tile_pool(name="pzp", bufs=1))
        pot = ctx.enter_context(tc.tile_pool(name="pot", bufs=2))
        pp = ctx.enter_context(tc.tile_pool(name="pp", bufs=4, space="PSUM"))

        fw2_t = const.tile([64, 2, 128], fp8, tag="m0")
        chsh_t = const.tile([128, 2, 256], fp8, tag="m1")
        zpaR_t = const.tile([128, 2, 128], fp8, tag="m2")
        zpbR_t = const.tile([128, 2, 128], fp8, tag="m3")
        gwa_t = const.tile([65, 128], bf16, tag="m4")
        gwb_t = const.tile([63, 128], bf16, tag="m5")
        ones_t = const.tile([1, 2, 128], fp8, tag="m6")
        nc.sync.dma_start(fw2_t[:], m_fw2[:])
        nc.sync.dma_start(chsh_t[:], m_chsh[:])
        nc.sync.dma_start(zpaR_t[:], m_zpa[:])
        nc.sync.dma_start(zpbR_t[:], m_zpb[:])
        nc.sync.dma_start(gwa_t[:], m_gwa[:])
        nc.sync.dma_start(gwb_t[:], m_gwb[:])
        nc.sync.dma_start(ones_t[:], m_ones[:])

        state = {}

        def dmaA(b):
            # weights for stages B (prefetch; wpool/bpool bufs=2 keeps them)
            w1r_t = wpool.tile([96, 2, 96], fp8, tag="w1r", name="w1r_t")
            w1i_t = wpool.tile([96, 2, 96], fp8, tag="w1i", name="w1i_t")
            w2_t = wpool.tile([96, 2, 192], fp8, tag="w2", name="w2_t")
            b2_t = wpool.tile([1, 2, 384], fp8, tag="b2", name="b2_t")
            nc.sync.dma_start(w1r_t[:], w1_ext[b, 0])
            nc.sync.dma_start(w1i_t[:], w1_ext[b, 1])
            nc.sync.dma_start(w2_t[:], w2_ext[b])
            nc.sync.dma_start(b2_t[:], b2_ext[b])
            b1r_t = bpool.tile([96, 1], f32, tag="b1r")
            b1i_t = bpool.tile([96, 1], f32, tag="b1i")
            nc.sync.dma_start(b1r_t[:], b1_ext[b, 0])
            nc.sync.dma_start(b1i_t[:], b1_ext[b, 1])
            # load x block: [64 wlo, c, whi, h] fp8
            xt = px.tile([64, 96, 2, 128], fp8, tag="X")
            nc.sync.dma_start(xt[:], xin[b])
            state[b] = (w1r_t, w1i_t, w2_t, b2_t, b1r_t, b1i_t)
            state[(b, "x")] = xt

        def stageA(b):
            (w1r_t, w1i_t, w2_t, b2_t, b1r_t, b1i_t) = state[b]
            xt = state.pop((b, "x"))

            # ---- S1: W-axis packed real FFT (DoubleRow over w halves) ----
            x1 = px1.tile([128, 2, 96, KPAD], fp8, tag="A")
            for g in range(12):             # 8 channels per 2-bank psum
                c0 = g * 8
                ps = pp.tile([128, 8, 128], f32, tag="ps")
                for j in range(8):
                    nc.tensor.matmul(ps[:, j, :], xt[:, c0 + j], fw2_t[:],
                                     start=True, stop=True, perf_mode=DR)
                nc.vector.tensor_copy(x1[:, 0, c0:c0 + 8, 0:65], ps[:, :, 0:65])
                nc.vector.tensor_copy(x1[:, 1, c0:c0 + 8, 1:64], ps[:, :, 65:128])
                yield
            # imag parts at kw=0,64 are zero
            nc.vector.memset(x1[:, 1, :, 0:1], 0.0)
            nc.vector.memset(x1[:, 1, :, 64:65], 0.0)

            # ---- S2: H-axis complex FFT (DR over re/im planes) ----
            y = py.tile([96, 2, WF, 128], fp8, tag="B")
            for g in range(17):             # 4 kw per 2-bank psum
                k0 = g * 4
                nk = min(4, WF - k0)
                ps = pp.tile([96, 4, 256], f32, tag="ps")
                for j in range(nk):
                    nc.tensor.matmul(ps[:, j, :], x1[:, :, :, k0 + j],
                                     chsh_t[:], start=True, stop=True,
                                     perf_mode=DR)
                nc.scalar.copy(y[:, 0, k0:k0 + nk, :], ps[:, 0:nk, 0:128])
                nc.scalar.copy(y[:, 1, k0:k0 + nk, :], ps[:, 0:nk, 128:256])
                yield
            state[(b, "y")] = y

        def stageB(b):
            (w1r_t, w1i_t, w2_t, b2_t, b1r_t, b1i_t) = state[b]
            y = state[(b, "y")]

            # ---- L1: block MLP layer 1 (DR over re/im input planes) ----
            o1 = po1.tile([96, 2, WF, 128], fp8, tag="C")
            for g in range(17):             # 4 kw per merged 2-bank psum
                ka = g * 4
                nk = min(4, WF - ka)
                ps = pp.tile([96, 8, 128], f32, tag="ps")
                rhs = y[:, :, ka:ka + nk, :]
                nc.tensor.matmul(ps[:, 0:nk, :], w1r_t[:], rhs,
                                 start=True, stop=True, perf_mode=DR)
                nc.tensor.matmul(ps[:, 4:4 + nk, :], w1i_t[:], rhs,
                                 start=True, stop=True, perf_mode=DR)
                nc.scalar.activation(o1[:, 0, ka:ka + nk, :], ps[:, 0:nk, :],
                                     RELU, bias=b1r_t[:])
                nc.scalar.activation(o1[:, 1, ka:ka + nk, :], ps[:, 4:4 + nk, :],
                                     RELU, bias=b1i_t[:])
                yield

            # ---- L2 (DR over re/im planes) + bias + softshrink ----
            z = pz.tile([128, 2, WF, 96], fp8, tag="D")
            for g in range(33):             # 2 kw per 1.5KB psum
                k0 = g * 2
                nk = min(2, WF - k0)
                ps = pp.tile([128, 2, 192], f32, tag="ps")
                nc.tensor.matmul(ps[:, 0:nk, :], ones_t[:],
                                 b2_t[:, :, 0:192 * nk],
                                 start=True, stop=False, perf_mode=DR)
                for j in range(nk):
                    nc.tensor.matmul(ps[:, j, :], o1[:, :, k0 + j, :], w2_t[:],
                                     start=False, stop=(j == nk - 1),
                                     perf_mode=DR, skip_group_check=True)
                cc = pcc.tile([128, 2, 192], bf16, tag="cc")
                nc.vector.tensor_scalar(cc[:, 0:nk, :], ps[:, 0:nk, :],
                                        -LAMS, LAMS, MAX, MIN)
                nc.vector.tensor_tensor(z[:, 0, k0:k0 + nk, :],
                                        ps[:, 0:nk, 0:96],
                                        cc[:, 0:nk, 0:96], SUB)
                nc.vector.tensor_tensor(z[:, 1, k0:k0 + nk, :],
                                        ps[:, 0:nk, 96:192],
                                        cc[:, 0:nk, 96:192], SUB)
                yield
            state[(b, "z")] = z

        def stageC(b):
            z = state.pop((b, "z"))
            state.pop((b, "y"))
            state.pop(b)

            # ---- invH (DR over re/im planes), partition-split zpA/zpB ----
            zpA = pzp.tile([65, 96, 128], fp8, tag="E")
            zpB = pzp.tile([63, 96, 128], fp8, tag="Eb")
            for g in range(24):             # 4 c per merged 2-bank psum
                c0 = g * 4
                ps = pp.tile([65, 8, 128], f32, tag="ps")
                for j in range(4):
                    cc0 = c0 + j
                    nc.tensor.matmul(ps[:, j, :], z[:, :, :, cc0], zpaR_t[:],
                                     start=True, stop=True, perf_mode=DR)
                    nc.tensor.matmul(ps[0:63, 4 + j, :], z[:, :, 1:64, cc0],
                                     zpbR_t[:], start=True, stop=True,
                                     perf_mode=DR)
                nc.scalar.copy(zpA[:, c0:c0 + 4, :], ps[:, 0:4, :])
                nc.vector.tensor_copy(zpB[:, c0:c0 + 4, :], ps[0:63, 4:8, :])
                yield

            # ---- invW (2-acc) + store ----
            ot = pot.tile([128, 96, 128], fp8, tag="F")
            for g in range(12):             # 8-c chunk per 2-bank psum
                c0 = g * 8
                ps = pp.tile([128, 8, 128], f32, tag="ps")
                for u in range(2):
                    ca = c0 + u * 4
                    nc.tensor.matmul(ps[:, 4 * u:4 * u + 4, :], gwa_t[:],
                                     zpA[:, ca:ca + 4, :],
                                     start=True, stop=False)
                    nc.tensor.matmul(ps[:, 4 * u:4 * u + 4, :], gwb_t[:],
                                     zpB[:, ca:ca + 4, :],
                                     start=False, stop=True)
                nc.scalar.copy(ot[:, c0:c0 + 8, :], ps[:])
                yield
            nc.sync.dma_start(oext[b], ot[:])

        def emit_all():
            for w in range(NB + 2):
                if w < NB:
                    dmaA(w)
                gens = []
                if w >= 2:
                    gens.append(stageC(w - 2))
                if 1 <= w <= NB:
                    gens.append(stageB(w - 1))
                if w < NB:
                    gens.append(stageA(w))
                while gens:
                    nxt = []
                    for gen in gens:
                        try:
                            next(gen)
                            nxt.append(gen)
                        except StopIteration:
                            pass
                    gens = nxt

        if rep > 1:
            with tc.For_i(0, rep, 1):
                emit_all()
        else:
            emit_all()

    nc.compile()
    return nc


_COMPILED = None


def _get_compiled():
    global _COMPILED
    if _COMPILED is None:
        _COMPILED = _build_graph()
    return _COMPILED


def _host_inputs(x, w1, b1, w2, b2):
    import ml_dtypes
    f8 = ml_dtypes.float8_e4m3
    bf = ml_dtypes.bfloat16
    fw, ch, sh, gw = _dft_mats()

    fw2 = fw.reshape(2, 64, 128).transpose(1, 0, 2)            # [64,2,128]
    chsh = np.stack([np.concatenate([ch, -sh], 1),
                     np.concatenate([sh, ch], 1)], 1)          # [128,2,256]
    zpaR = np.stack([ch, -sh], 1)                              # [128,2,128]
    zpbR = np.stack([sh, ch], 1)
    onesdr = np.stack([np.ones((1, 128)), np.zeros((1, 128))], 1)

    w1r = np.stack([SC1 * w1[0], -SC1 * w1[1]], 2)             # [NB,96,2,96]
    w1i = np.stack([SC1 * w1[1], SC1 * w1[0]], 2)
    w1dr = np.stack([w1r, w1i], 1)                             # [NB,2,96,2,96]
    w2p0 = np.concatenate([SC2 * w2[0], SC2 * w2[1]], 2)       # [NB,96,192]
    w2p1 = np.concatenate([-SC2 * w2[1], SC2 * w2[0]], 2)
    w2dr = np.stack([w2p0, w2p1], 2)                           # [NB,96,2,192]
    b2cat = np.concatenate([SCL2 * b2[0], SCL2 * b2[1],
                            SCL2 * b2[0], SCL2 * b2[1]], 1)    # [NB,384]
    b2dr = np.stack([b2cat, np.zeros_like(b2cat)], 1)[:, None] # [NB,1,2,384]
    b1s = (SC1 * b1.transpose(1, 0, 2))[:, :, :, None]         # [NB,2,96,1]

    common = {
        "fw2": fw2.astype(f8), "chsh": chsh.astype(f8),
        "zpaR": zpaR.astype(f8), "zpbR": zpbR.astype(f8),
        "gwA": (gw[0:65] * (OSCALE / SCL2)).astype(bf),
        "gwB": (gw[65:128] * (OSCALE / SCL2)).astype(bf),
        "onesdr": onesdr.astype(f8),
        "w1dr": np.ascontiguousarray(w1dr.reshape(NB, 2, 96, 2, 96)).astype(f8),
        "w2dr": np.ascontiguousarray(w2dr).astype(f8),
        "b2dr": np.ascontiguousarray(b2dr).astype(f8),
        "b1s": np.ascontiguousarray(b1s).astype(np.float32),
    }
    in_maps = []
    for i in range(NCORES):
        # [h,w,c] -> [w,c,h] -> [whi,wlo,c,h] -> [wlo,c,whi,h]
        xi = np.asarray(x[i]).transpose(1, 2, 0).reshape(2, 64, C, H)
        xi = np.ascontiguousarray(xi.transpose(1, 2, 0, 3))    # [64, C, 2, 128]
        xi = xi.reshape(64, NB, 96, 2, 128).transpose(1, 0, 2, 3, 4)
        m = dict(common)
        m["x"] = np.ascontiguousarray(xi).astype(f8)
        in_maps.append(m)
    return in_maps


def kernel(x, w1, b1, w2, b2, _trace=False):
    from concourse.bass_utils import run_bass_kernel_spmd

    nc = _get_compiled()
    in_maps = _host_inputs(x, w1, b1, w2, b2)
    res = run_bass_kernel_spmd(nc, in_maps, core_ids=list(range(NCORES)),
                               trace=_trace)
    y = np.empty((NCORES, H, W, C), dtype=np.float32)
    for i in range(NCORES):
        o = np.asarray(res.results[i]["out"]).astype(np.float32) / OSCALE
        # o[nb, w, c, h] -> y[h, w, nb, c]
        o = o.reshape(NB, W, BS, H).transpose(3, 1, 0, 2).reshape(H, W, C)
        y[i] = o + np.asarray(x[i])
    if _trace:
        return y, res
    return y


def _bench_nc(nc, inputs, iters=10):
    """Min wall-clock (ns) of the jitted sharded call for a prebuilt graph."""
    import time
    import jax
    from jax.sharding import Mesh, PartitionSpec
    from jax.experimental.shard_map import shard_map
    from concourse import bass2jax, mybir

    bass2jax.install_neuronx_cc_hook()
    in_maps = _host_inputs(inputs["x"], inputs["w1"], inputs["b1"],
                           inputs["w2"], inputs["b2"])

    pname = nc.partition_id_tensor.name if nc.partition_id_tensor else None
    in_names, out_names, out_avals, zero_outs = [], [], [], []
    for alloc in nc.m.functions[0].allocations:
        if not isinstance(alloc, mybir.MemoryLocationSet):
            continue
        name = alloc.memorylocations[0].name
        if alloc.kind == "ExternalInput":
            if name != pname:
                in_names.append(name)
        elif alloc.kind == "ExternalOutput":
            shape = tuple(alloc.tensor_shape)
            dtype = mybir.dt.np(alloc.dtype)
            out_names.append(name)
            out_avals.append(jax.core.ShapedArray(shape, dtype))
            zero_outs.append(np.zeros(shape, dtype))
    n_params = len(in_names)
    all_names = in_names + out_names
    if pname is not None:
        all_names = all_names + [pname]

    def _body(*args):
        operands = list(args)
        if pname is not None:
            operands.append(bass2jax.partition_id_tensor())
        outs = bass2jax._bass_exec_p.bind(
            *operands, out_avals=tuple(out_avals), in_names=tuple(all_names),
            out_names=tuple(out_names), lowering_input_output_aliases=(),
            sim_require_finite=True, sim_require_nnan=True, nc=nc)
        return tuple(outs)

    devices = jax.devices()[:NCORES]
    mesh = Mesh(np.asarray(devices), ("core",))
    nops = n_params + len(out_names)
    sharded = jax.jit(shard_map(_body, mesh=mesh,
                                in_specs=(PartitionSpec("core"),) * nops,
                                out_specs=(PartitionSpec("core"),) * len(out_names),
                                check_rep=False), keep_unused=True)
    concat_in = [np.concatenate([np.asarray(in_maps[c][n]) for c in range(NCORES)], axis=0)
                 for n in in_names]
    concat_zero = [np.zeros((NCORES * z.shape[0], *z.shape[1:]), z.dtype) for z in zero_outs]
    sharding = jax.sharding.NamedSharding(mesh, PartitionSpec("core"))
    dev_in = [jax.device_put(a, sharding) for a in concat_in + concat_zero]
    for _ in range(2):
        r = sharded(*dev_in)
        jax.block_until_ready(r)
    best = float("inf")
    for _ in range(iters):
        t0 = time.perf_counter()
        r = sharded(*dev_in)
        jax.block_until_ready(r)
        best = min(best, time.perf_counter() - t0)
    return best * 1e9


def bench(inputs, iters=10, rep=17):
    """Estimate HW exec time via on-device repeat loop slope."""
    t1 = _bench_nc(_get_compiled(), inputs, iters)
    ncr = _build_graph(rep=rep)
    tr = _bench_nc(ncr, inputs, iters)
    print(f"  [bench] T(1)={t1/1e6:.2f} ms  T({rep})={tr/1e6:.2f} ms")
    return (tr - t1) / (rep - 1)


if __name__ == "__main__":
    nc = _get_compiled()
    print("graph built + compiled OK")
